# revision 16
# baseline (speedup 1.0000x reference)
"""Trainium2 Bass kernel for the Per-Region-Encoding model.

Contract: kernel(**inputs) takes the FULL unsharded inputs
  input        [8, 256, 32, 32] f32
  segmentation [8, 8, 256, 256] f32
  params       dict of conv/norm parameters
and returns (codes_vector [8,9,256], exist_vector [8,8], x [8,256,128,128]).

Strategy: pure data parallel over batch -- one sample per NeuronCore, 8 cores.
Per core the whole net runs on-chip:
  - 3x3 convs as implicit GEMM (9 shifted matmuls accumulating in PSUM), bf16.
  - stride-2 transposed convs decomposed into 4 parity quarters (1/2/2/4 taps);
    main + shortcut convT accumulate into the same PSUM chunk.
  - InstanceNorm stats via bn_stats/bn_aggr on the fp32 PSUM chunks of the
    producing conv (bias folded into the mean), normalize+LeakyReLU fused into
    one ScalarE Prelu activation.
  - codes = tanh(conv1x1) computed pixel-major so the segment reduction is a
    plain matmul against the transposed one-hot seg mask [pix, 9] (9th col of
    ones yields the global-mean row); counts via a ones-column matmul.
"""

import os
import sys

for _p in ("/opt/trn_rl_repo", "/root/.axon_site/_ro/trn_rl_repo"):
    if os.path.isdir(_p) and _p not in sys.path:
        sys.path.append(_p)

import numpy as np
import ml_dtypes

import concourse.bass as bass
import concourse.tile as tile
from concourse import bacc, mybir
from concourse.masks import make_identity

bfnp = ml_dtypes.bfloat16
f32 = mybir.dt.float32
bf16 = mybir.dt.bfloat16
AF = mybir.ActivationFunctionType

N_CORES = 8
C = 256
EPS = 1e-5
SIM_COMPAT = bool(int(os.environ.get("KERNEL_SIM_COMPAT", "0")))

# convT quarter -> list of (offset dy*3+dx, da, db)
TAPS = {
    (0, 0): [(4, 0, 0)],
    (0, 1): [(3, 0, 0), (5, 0, 1)],
    (1, 0): [(1, 0, 0), (7, 1, 0)],
    (1, 1): [(0, 0, 0), (2, 0, 1), (6, 1, 0), (8, 1, 1)],
}
QUARTERS = [(0, 0), (0, 1), (1, 0), (1, 1)]

_CACHE = {}


def _build(reps=None):
    nc = bacc.Bacc("TRN2", target_bir_lowering=False, debug=False,
                   num_devices=N_CORES)

    xin = nc.dram_tensor("xin", [C, 32, 32], f32, kind="ExternalInput")
    seg = nc.dram_tensor("seg", [8, 256, 256], f32, kind="ExternalInput")
    wdr = {}
    for b in range(2):
        for nm in ("c", "t", "s"):
            wdr[(nm, b)] = nc.dram_tensor(f"w_{nm}{b}", [2, 128, 9, 2, 128],
                                          bf16, kind="ExternalInput")
    cw = nc.dram_tensor("cw", [2, 128, 256], bf16, kind="ExternalInput")
    cb = nc.dram_tensor("cb", [1, 256], bf16, kind="ExternalInput")
    vec = {b: nc.dram_tensor(f"vec{b}", [128, 2, 6], f32, kind="ExternalInput")
           for b in range(2)}

    xout = nc.dram_tensor("xout", [C, 128, 128], f32, kind="ExternalOutput")
    cvout = nc.dram_tensor("cv", [9, 256], f32, kind="ExternalOutput")
    evout = nc.dram_tensor("ev", [8, 1], f32, kind="ExternalOutput")

    with tile.TileContext(nc) as tc:
        _body(nc, tc, xin, seg, wdr, cw, cb, vec, xout, cvout, evout, reps=reps)
    nc.compile()
    return nc


def _memset_border2(nc, t, H, W):
    """Zero the 1-wide border of a [128, H+2, W+2] padded tile."""
    nc.vector.memset(t[:, 0, :], 0.0)
    nc.vector.memset(t[:, H + 1, :], 0.0)
    nc.vector.memset(t[:, 1:H + 1, 0:1], 0.0)
    nc.vector.memset(t[:, 1:H + 1, W + 1:W + 2], 0.0)


def _memset_border1(nc, t, H, W):
    """Zero the bottom row / right col of a [128, H+1, W+1] padded tile."""
    nc.vector.memset(t[:, H, :], 0.0)
    nc.vector.memset(t[:, 0:H, W:W + 1], 0.0)


DEFAULT_REPS = int(os.environ.get("KERNEL_REPS", "1"))


def _body(nc, tc, xin, seg, wdr, cw, cb, vec, xout, cvout, evout, reps=None):
    for _rep in range(reps or DEFAULT_REPS):
        _body_once(nc, tc, xin, seg, wdr, cw, cb, vec, xout, cvout, evout)


def _body_once(nc, tc, xin, seg, wdr, cw, cb, vec, xout, cvout, evout):
    from contextlib import ExitStack
    ctx = ExitStack()
    with ctx:
        consts = ctx.enter_context(tc.tile_pool(name="consts", bufs=1))
        ps_main = ctx.enter_context(tc.tile_pool(name="ps_main", bufs=4, space="PSUM"))
        ps_code = ctx.enter_context(tc.tile_pool(name="ps_code", bufs=2, space="PSUM"))
        ps_acc = ctx.enter_context(tc.tile_pool(name="ps_acc", bufs=1, space="PSUM"))
        stat_pool = ctx.enter_context(tc.tile_pool(name="stats", bufs=1))
        sm_pool = ctx.enter_context(tc.tile_pool(name="small", bufs=4))
        tmp_pool = ctx.enter_context(tc.tile_pool(name="tmp", bufs=1)) \
            if SIM_COMPAT else None

        # ---- critical-path inputs first: sample data, then block-0 weights
        x0_32 = consts.tile([128, 2, 1024], f32)
        nc.sync.dma_start(out=x0_32,
                          in_=xin.rearrange("(a b) h w -> b a (h w)", a=2))

        # weights: block-0 first (needed early), block-1 deferred until
        # block-0 work has been emitted (scheduler priority follows order)
        wsb = {}

        def load_w(key):
            t = consts.tile([128, 2, 9, 2, 128], bf16, tag=f"w_{key[0]}{key[1]}",
                            name=f"w_{key[0]}{key[1]}")
            for kt in range(2):
                nc.sync.dma_start(out=t[:, kt], in_=wdr[key][kt])
            wsb[key] = t

        for key in (("c", 0), ("s", 0), ("t", 0)):
            load_w(key)
        vsb = {}
        for b in range(2):
            vt = consts.tile([128, 2, 6], f32, tag=f"vec{b}", name=f"vecs{b}")
            nc.sync.dma_start(out=vt, in_=vec[b][:])
            vsb[b] = vt
        cwt = consts.tile([128, 2, 256], bf16)
        for kt in range(2):
            nc.sync.dma_start(out=cwt[:, kt], in_=cw[kt])
        cbt = consts.tile([1, 256], bf16)
        nc.sync.dma_start(out=cbt, in_=cb[:])
        ones_k1 = consts.tile([1, 128], bf16)
        nc.vector.memset(ones_k1, 1.0)
        ones_col = consts.tile([128, 1], bf16)
        nc.vector.memset(ones_col, 1.0)
        epst = consts.tile([128, 1], f32)
        nc.vector.memset(epst, EPS)
        alphat = consts.tile([128, 1], f32)
        nc.vector.memset(alphat, 0.1)

        # helpers ------------------------------------------------------------
        def prelu(out_ap, in_ap, scale_ap, bias_ap):
            """out = lrelu(in*scale + bias), slope 0.1."""
            if not SIM_COMPAT:
                nc.scalar.activation(out=out_ap, in_=in_ap, func=AF.Prelu,
                                     bias=bias_ap, scale=scale_ap, alpha=alphat)
            else:
                shp = [128] + list(in_ap.shape[1:])
                t = tmp_pool.tile(shp, f32, tag="prelu_tmp", name="prelu_tmp")
                nc.vector.tensor_scalar(out=t, in0=in_ap, scalar1=scale_ap,
                                        scalar2=bias_ap,
                                        op0=mybir.AluOpType.mult,
                                        op1=mybir.AluOpType.add)
                nc.vector.scalar_tensor_tensor(out=out_ap, in0=t, scalar=0.1,
                                               in1=t,
                                               op0=mybir.AluOpType.mult,
                                               op1=mybir.AluOpType.max)

        def mk_scale_bias(mv, g_ap, b_ap, extra_mean_ap, tag):
            """scale = g/sqrt(var+eps); bias = b - (mean+extra)*scale."""
            rs = sm_pool.tile([128, 1], f32, tag=f"rs_{tag}", name=f"rs_{tag}")
            nc.scalar.activation(out=rs, in_=mv[:, 1:2], func=AF.Sqrt,
                                 bias=epst, scale=1.0)
            nc.vector.reciprocal(rs, rs)
            sc = sm_pool.tile([128, 1], f32, tag=f"sc_{tag}", name=f"sc_{tag}")
            nc.vector.tensor_mul(sc, rs, g_ap)
            mean = sm_pool.tile([128, 1], f32, tag=f"mean_{tag}",
                                name=f"mean_{tag}")
            if extra_mean_ap is not None:
                nc.vector.tensor_add(mean, mv[:, 0:1], extra_mean_ap)
            else:
                nc.vector.tensor_copy(out=mean, in_=mv[:, 0:1])
            bi = sm_pool.tile([128, 1], f32, tag=f"bi_{tag}", name=f"bi_{tag}")
            nc.vector.tensor_mul(bi, mean, sc)
            nc.vector.tensor_sub(bi, b_ap, bi)
            return sc, bi

        def aggr(stats_tile, kt, tag):
            mv = sm_pool.tile([128, 2], f32, tag=f"mv_{tag}", name=f"mv_{tag}")
            nc.vector.bn_aggr(out=mv, in_=stats_tile[:, kt])
            return mv

        # --------------------------------------------------------------------
        def res_block(bi_, H, W, xpad, norm_views, stats_in, stats_in_extra,
                      out_writer, blk_pool):
            """xpad: list of 2 padded raw-input tiles [128,H+2,W+2] bf16
            (interior at +1,+1). norm_views[kt]: AP [128,H,W] raw input for
            the first normalize. stats_in: [128,2,n,6] bn_stats of the raw
            input. stats_in_extra: APs added to the input mean or None.
            out_writer(mt, py, px, band, rpc, psum)."""
            v = vsb[bi_]
            HW = H * W
            rpc = 512 // W
            nch = H // rpc

            # padded normalized tiles, split per kt so each conv matmul only
            # waits on the half it reads
            t_pad = [blk_pool.tile([128, H + 2, W + 2], bf16,
                                   tag=f"tpad{bi_}_{kt}", name=f"tpad{bi_}_{kt}")
                     for kt in range(2)]
            t2_pad = [blk_pool.tile([128, H + 1, W + 1], bf16,
                                    tag=f"t2pad{bi_}_{kt}", name=f"t2pad{bi_}_{kt}")
                      for kt in range(2)]
            for kt in range(2):
                _memset_border2(nc, t_pad[kt], H, W)
                _memset_border1(nc, t2_pad[kt], H, W)

            for kt in range(2):
                mv = aggr(stats_in, kt, f"i1_{bi_}_{kt}")
                extra = stats_in_extra[kt] if stats_in_extra is not None else None
                sc, bi2 = mk_scale_bias(mv, v[:, kt, 0:1], v[:, kt, 1:2],
                                        extra, f"i1_{bi_}_{kt}")
                prelu(t_pad[kt][:, 1:H + 1, 1:W + 1], norm_views[kt], sc, bi2)

            # conv3x3 -> h (+ psum stats for inorm2), h split per mt
            h = [blk_pool.tile([128, HW], bf16, tag=f"h{bi_}_{mt}",
                               name=f"h{bi_}_{mt}") for mt in range(2)]
            stats_h = stat_pool.tile([128, 2, nch, 6], f32, tag=f"sh{bi_}",
                                     name=f"sh{bi_}")
            wc = wsb[("c", bi_)]
            for mt in range(2):
                for ch in range(nch):
                    r0 = ch * rpc
                    psum = ps_main.tile([128, 512], f32, tag="psum", name="psum")
                    idx = 0
                    for kt in range(2):
                        for off in range(9):
                            dy, dx = divmod(off, 3)
                            nc.tensor.matmul(
                                psum,
                                wc[:, kt, off, mt, :],
                                t_pad[kt][:, dy + r0:dy + r0 + rpc, dx:dx + W],
                                start=(idx == 0), stop=(idx == 17))
                            idx += 1
                    nc.vector.bn_stats(out=stats_h[:, mt, ch, :], in_=psum)
                    nc.scalar.activation(out=h[mt][:, r0 * W:r0 * W + 512],
                                         in_=psum, func=AF.Identity,
                                         bias=v[:, mt, 4:5], scale=1.0)

            for kt in range(2):
                mv = aggr(stats_h, kt, f"i2_{bi_}_{kt}")
                sc, bi2 = mk_scale_bias(mv, v[:, kt, 2:3], v[:, kt, 3:4],
                                        v[:, kt, 4:5], f"i2_{bi_}_{kt}")
                prelu(t2_pad[kt][:, 0:H, 0:W],
                      h[kt].rearrange("p (h w) -> p h w", h=H), sc, bi2)

            # convT: main (w_t on t2_pad) + shortcut (w_s on xpad)
            wt = wsb[("t", bi_)]
            ws = wsb[("s", bi_)]
            for band in range(nch):
                r0 = band * rpc
                for mt in range(2):
                    for py, px in QUARTERS:
                        taps = TAPS[(py, px)]
                        total = len(taps) * 4
                        psum = ps_main.tile([128, rpc, W], f32, tag="psum",
                                            name="psum")
                        idx = 0
                        for w_, src, ofs in ((ws, xpad, 1), (wt, t2_pad, 0)):
                            for off, da, db in taps:
                                for kt in range(2):
                                    nc.tensor.matmul(
                                        psum,
                                        w_[:, kt, off, mt, :],
                                        src[kt][:,
                                                ofs + r0 + da:ofs + r0 + da + rpc,
                                                ofs + db:ofs + db + W],
                                        start=(idx == 0), stop=(idx == total - 1))
                                    idx += 1
                        out_writer(mt, py, px, band, rpc, psum)

        # ---- block 0 ----
        stats_x0 = stat_pool.tile([128, 2, 2, 6], f32, tag="sx0")
        for kt in range(2):
            for i in range(2):
                nc.vector.bn_stats(out=stats_x0[:, kt, i, :],
                                   in_=x0_32[:, kt, 512 * i:512 * i + 512])

        xp1_pool = ctx.enter_context(tc.tile_pool(name="xp1", bufs=1))
        x1_pad = [xp1_pool.tile([128, 66, 66], bf16, tag=f"x1pad_{kt}",
                                name=f"x1pad_{kt}") for kt in range(2)]
        for kt in range(2):
            _memset_border2(nc, x1_pad[kt], 64, 64)

        blk0tmp_cm = tc.tile_pool(name="blk0tmp", bufs=1)
        blk0tmp = blk0tmp_cm.__enter__()
        x0_pad = [blk0tmp.tile([128, 34, 34], bf16, tag=f"x0pad_{kt}",
                               name=f"x0pad_{kt}") for kt in range(2)]
        for kt in range(2):
            _memset_border2(nc, x0_pad[kt], 32, 32)
            nc.vector.tensor_copy(
                out=x0_pad[kt][:, 1:33, 1:33],
                in_=x0_32[:, kt, :].rearrange("p (h w) -> p h w", h=32))

        stats_x1 = stat_pool.tile([128, 2, 8, 6], f32, tag="sx1")
        _q_idx = [0, 0]

        def writer0(mt, py, px, band, rpc, psum):
            r0 = band * rpc
            nc.scalar.activation(
                out=x1_pad[mt][:,
                               1 + 2 * r0 + py:1 + 2 * r0 + py + 2 * rpc:2,
                               1 + px:1 + px + 64:2],
                in_=psum, func=AF.Identity,
                bias=vsb[0][:, mt, 5:6], scale=1.0)
            nc.vector.bn_stats(out=stats_x1[:, mt, _q_idx[mt], :],
                               in_=psum.rearrange("p a b -> p (a b)"))
            _q_idx[mt] += 1

        res_block(0, 32, 32, x0_pad,
                  [x0_32[:, kt, :].rearrange("p (h w) -> p h w", h=32)
                   for kt in range(2)],
                  stats_x0, None, writer0, blk0tmp)
        blk0tmp_cm.__exit__(None, None, None)

        # ---- block-1 weights (needed ~1/4 into the kernel) ----
        for key in (("s", 1), ("c", 1), ("t", 1)):
            load_w(key)

        # ---- seg downsample + transpose to [pix, s] (needed by block 1) ----
        ident = consts.tile([128, 128], f32)
        make_identity(nc, ident)
        segT = consts.tile([128, 128, 9], bf16)
        nc.vector.memset(segT[:, :, 8:9], 1.0)

        with tc.tile_pool(name="segload", bufs=2) as segload:
            for s in range(4):
                slab = segload.tile([8, 32, 256], f32, tag="slab")
                nc.sync.dma_start(out=slab,
                                  in_=seg[:, 64 * s:64 * s + 64:2, :])
                pst = ps_main.tile([128, 32, 8], f32, tag="psum", name="pst")
                for cl in range(32):
                    nc.tensor.transpose(out=pst[:, cl, :],
                                        in_=slab[:, cl, 0:256:2],
                                        identity=ident[:8, :8])
                nc.vector.tensor_copy(out=segT[:, 32 * s:32 * s + 32, 0:8], in_=pst)

        # ---- block 1 (+ fused codes/segment reduction per band) ----
        blk1 = ctx.enter_context(tc.tile_pool(name="blk1", bufs=1))
        stg_pool = ctx.enter_context(tc.tile_pool(name="stg", bufs=3))
        x2b_pool = ctx.enter_context(tc.tile_pool(name="x2b", bufs=2))
        code_pool = ctx.enter_context(tc.tile_pool(name="code", bufs=1))

        psum_sc = ps_acc.tile([9, 257], f32)
        cods = []
        for ci_ in range(3):
            codt = code_pool.tile([128, 257], bf16, tag=f"cod{ci_}",
                                  name=f"cod{ci_}")
            nc.vector.memset(codt[:, 256:257], 1.0)
            cods.append(codt)

        band_state = {}

        def writer1(mt, py, px, band, rpc, psum):
            st = band_state.get(band)
            if st is None:
                st = {"stg": {}, "done": 0, "x2b": {}}
                band_state[band] = st
            if mt not in st["stg"]:
                st["stg"][mt] = stg_pool.tile([128, 16, 128], f32, tag="stg",
                                              name=f"stg_{band}_{mt}")
            stg = st["stg"][mt]
            nc.scalar.activation(
                out=stg[:, py:16:2, px:128:2], in_=psum,
                func=AF.Identity, bias=vsb[1][:, mt, 5:6], scale=1.0)
            st["done"] += 1
            if st["done"] % 4 == 0:
                # quarter set for this mt complete: ship rows + cast to bf16
                nc.sync.dma_start(
                    out=xout[mt * 128:(mt + 1) * 128,
                             band * 16:band * 16 + 16, :],
                    in_=stg)
                x2b = x2b_pool.tile([128, 2048], bf16, tag=f"x2b_{mt}",
                                    name=f"x2b_{band}_{mt}")
                st["x2b"][mt] = x2b
                nc.vector.tensor_copy(
                    out=x2b, in_=stg.rearrange("p a b -> p (a b)"))
            if st["done"] == 8:
                # both halves done: codes + segment reduce for these 16 rows
                for cl in range(16):
                    c = band * 16 + cl
                    psc = ps_code.tile([128, 256], f32, tag="psc", name="psc")
                    for kt in range(2):
                        nc.tensor.matmul(psc,
                                         st["x2b"][kt][:, cl * 128:(cl + 1) * 128],
                                         cwt[:, kt, :],
                                         start=(kt == 0), stop=False)
                    nc.tensor.matmul(psc, ones_k1, cbt, start=False, stop=True)
                    cod = cods[c % 3]
                    nc.scalar.activation(out=cod[:, 0:256], in_=psc, func=AF.Tanh)
                    nc.tensor.matmul(psum_sc, segT[:, c, :], cod,
                                     start=(c == 0), stop=(c == 127))

        res_block(1, 64, 64, x1_pad,
                  [x1_pad[kt][:, 1:65, 1:65] for kt in range(2)],
                  stats_x1, [vsb[0][:, kt, 5:6] for kt in range(2)],
                  writer1, blk1)

        # ---- epilogue: means / exist ----
        cnt32 = sm_pool.tile([9, 1], f32, tag="cnt")
        nc.vector.tensor_copy(out=cnt32, in_=psum_sc[:, 256:257])
        cmax = sm_pool.tile([9, 1], f32, tag="cmax")
        nc.vector.tensor_scalar_max(cmax, cnt32, 1.0)
        inv = sm_pool.tile([9, 1], f32, tag="inv")
        nc.vector.reciprocal(inv, cmax)
        cv_sb = sm_pool.tile([9, 256], f32, tag="cv")
        nc.scalar.activation(out=cv_sb, in_=psum_sc[:, 0:256], func=AF.Identity,
                             scale=inv)
        nc.sync.dma_start(out=cvout[:], in_=cv_sb)
        ev_sb = sm_pool.tile([8, 1], f32, tag="ev")
        nc.vector.tensor_scalar_min(ev_sb, cnt32[0:8, :], 1.0)
        nc.sync.dma_start(out=evout[:], in_=ev_sb)


# -------------------------------------------------------------------------
# host side
# -------------------------------------------------------------------------

def _pack_conv(w):
    """[co, ci, 3, 3] f32 -> [2(kt), 128, 9, 2(mt), 128] bf16 (lhsT layout)."""
    w = np.asarray(w, np.float32)
    arr = np.empty((2, 128, 9, 2, 128), bfnp)
    for off in range(9):
        dy, dx = divmod(off, 3)
        for kt in range(2):
            for mt in range(2):
                arr[kt, :, off, mt, :] = \
                    w[mt * 128:(mt + 1) * 128, kt * 128:(kt + 1) * 128, dy, dx] \
                    .T.astype(bfnp)
    return arr


def _pack_vec(p):
    """per-channel vectors -> [128, 2, 6] f32 (g1, b1n, g2, b2n, b1, b2+bs)."""
    cols = [np.asarray(p['in1_g'], np.float32),
            np.asarray(p['in1_b'], np.float32),
            np.asarray(p['in2_g'], np.float32),
            np.asarray(p['in2_b'], np.float32),
            np.asarray(p['b1'], np.float32),
            np.asarray(p['b2'], np.float32) + np.asarray(p['bs'], np.float32)]
    arr = np.empty((128, 2, 6), np.float32)
    for j, c in enumerate(cols):
        arr[:, 0, j] = c[:128]
        arr[:, 1, j] = c[128:]
    return arr


def _pack_inputs(input, segmentation, params):
    x = np.ascontiguousarray(np.asarray(input, np.float32))
    segm = np.ascontiguousarray(np.asarray(segmentation, np.float32))
    shared = {}
    for b in range(2):
        p = params[f'blk{b}']
        shared[f"w_c{b}"] = _pack_conv(p['w1'])
        shared[f"w_t{b}"] = _pack_conv(p['w2'])
        shared[f"w_s{b}"] = _pack_conv(p['ws'])
        shared[f"vec{b}"] = _pack_vec(p)
    cwn = np.asarray(params['code_w'], np.float32)[:, :, 0, 0]  # [co, ci]
    cw_arr = np.empty((2, 128, 256), bfnp)
    for kt in range(2):
        cw_arr[kt] = cwn[:, kt * 128:(kt + 1) * 128].T.astype(bfnp)
    shared["cw"] = cw_arr
    shared["cb"] = np.asarray(params['code_b'], np.float32)[None, :].astype(bfnp)
    return [dict(shared, xin=x[b], seg=segm[b]) for b in range(x.shape[0])]


def kernel(input, segmentation, params):
    from concourse.bass_utils import run_bass_kernel_spmd

    if "nc" not in _CACHE:
        _CACHE["nc"] = _build()
    nc = _CACHE["nc"]

    in_maps = _pack_inputs(input, segmentation, params)
    res = run_bass_kernel_spmd(nc, in_maps, list(range(N_CORES)))
    _CACHE["last_result"] = res

    B = len(in_maps)
    cv = np.stack([res.results[b]["cv"] for b in range(B)])
    ev = np.stack([res.results[b]["ev"][:, 0] for b in range(B)])
    xo = np.stack([res.results[b]["xout"] for b in range(B)])
    return cv.astype(np.float32), ev.astype(np.float32), xo.astype(np.float32)
